# revision 13
# baseline (speedup 1.0000x reference)
"""Trainium2 Bass kernel for nn_HODE_MDP (hypergraph ODE message passing).

Math (T_UP = T_GEO = T_P2P = 1.0, ALPHA = 0.8):
    pe  = poi_emb_weight[:-1]                      # [P, D]
    x/s/g = pe * sigmoid(pe @ W_t + b_t)           # col / seq / geo gates
    hg_pois    = x + HG_pu @ (HG_up @ x)
    geo_pois   = g + 0.4 * (poi_geo_graph @ g)
    trans_pois = s + HG_poi_src @ (HG_poi_tar @ s)
    hg_users   = (HG_up @ hg_pois)[user_idx]
    geo_users  = (HG_up @ geo_pois)[user_idx]
    out = concat([hg_pois, geo_pois, trans_pois, hg_users, geo_users])

Distribution (8 NeuronCores), v3 — everything contract-dim sharded, all
collectives are ReduceScatters (measured ~3x cheaper than AllReduce on
this stack's DRAM-bounce mesh):
  * gates are computed for the core's own 1024-row block only.
  * stage 1: y_up|y_tar partials ([D, U+E]) from local x/s blocks
    against local row-blocks of Up^T/Tar^T; fused RS1 leaves each core
    the y rows it needs for stage 2 (Pu/Src column-block sharding).
  * stage 2: hg/trans delta partials ([D, P] each) from the y slice
    against column-blocks of Pu/Src; geo delta partial from the local
    g block against the row-block of Geo^T; all three fuse into RS2
    ([D, 3P] -> own [D, 3*PP] row-slices, summed).
  * user embeddings: host pre-gathers Up[user_idx] -> Sel [B, P];
    users = Sel@x + Sel@delta.  The base term runs during stage 1 off
    local gates; the delta term after RS2; the HOST sums the 8 partial
    [D, 2B] outputs (no device collective).

All big streams are fp8 e4m3 with power-of-two pre-scaling (host side)
and DoubleRow matmuls (2 k-tiles per instruction = 2x PE throughput).
The delta terms are ~3e-3 of the residual scale, so fp8 on the delta
paths costs ~6e-5 relative error; the user matmuls run in bf16.
"""

import sys

if "/opt/trn_rl_repo" not in sys.path:
    sys.path.insert(0, "/opt/trn_rl_repo")

import numpy as np
import ml_dtypes

import concourse.bass as bass  # noqa: F401
import concourse.bacc as bacc
import concourse.mybir as mybir
import concourse.tile as tile
from concourse.bass_utils import run_bass_kernel_spmd

F32 = mybir.dt.float32
BF16 = mybir.dt.bfloat16
FP8 = mybir.dt.float8e4
SIG = mybir.ActivationFunctionType.Sigmoid
COPY = mybir.ActivationFunctionType.Copy
MULT = mybir.AluOpType.mult
ADD = mybir.AluOpType.add
DR = mybir.MatmulPerfMode.DoubleRow

NCORES = 8
P, U, E, D, B = 8192, 4096, 4096, 128, 1024
PP = P // NCORES            # 1024 P-rows per core
UU = U // NCORES            # 512
EE = E // NCORES            # 512
KL = PP // 128              # 8 local k-tiles
KY = (UU + EE) // 128       # 8 k-tiles in the y slice
RG = [list(range(NCORES))]

SX = 128.0                  # gate -> fp8 scale (2^7)
SAR = 2.0 ** -7             # psum -> RS payload scale
S_DELTA = 2.0 ** -28        # RS2 hg/tr segment -> f32 delta
S_GEO = 0.4 * 2.0 ** -18    # RS2 geo segment (2^18 * Geo@g) -> delta

_CACHE: dict = {}


def _build_nc():
    nc = bacc.Bacc(
        "TRN2",
        target_bir_lowering=False,
        debug=False,
        enable_asserts=False,
        num_devices=NCORES,
    )

    # ---- per-core DRAM I/O ----------------------------------------------
    peT_bf = nc.dram_tensor("peT_bf", [D, PP], BF16, kind="ExternalInput").ap()
    peT_f = nc.dram_tensor("peT_f", [D, PP], F32, kind="ExternalInput").ap()
    w3 = nc.dram_tensor("w3", [D, 3, D], BF16, kind="ExternalInput").ap()
    bT3 = nc.dram_tensor("bT3", [D, 3], F32, kind="ExternalInput").ap()
    ident_f = nc.dram_tensor("ident_f", [D, D], F32, kind="ExternalInput").ap()
    ident_b = nc.dram_tensor("ident_b", [D, D], BF16, kind="ExternalInput").ap()
    # row-blocks of the transposed matrices (contract-dim shards), fp8
    UpT = nc.dram_tensor("UpT", [PP, U], FP8, kind="ExternalInput").ap()
    TarT = nc.dram_tensor("TarT", [PP, E], FP8, kind="ExternalInput").ap()
    GeoT = nc.dram_tensor("GeoT", [PP, P], FP8, kind="ExternalInput").ap()
    # column-block shards for stage 2: rows = own y slice
    PuT = nc.dram_tensor("PuT", [UU, P], FP8, kind="ExternalInput").ap()
    SrcT = nc.dram_tensor("SrcT", [EE, P], FP8, kind="ExternalInput").ap()
    SelT = nc.dram_tensor("SelT", [PP, B], BF16, kind="ExternalInput").ap()

    poisT_o = nc.dram_tensor("poisT_o", [3, D, PP], F32, kind="ExternalOutput").ap()
    usersT_o = nc.dram_tensor("usersT_o", [D, 2 * B], F32, kind="ExternalOutput").ap()

    with tile.TileContext(nc) as tc:
        with (
            tc.tile_pool(name="const", bufs=1) as constp,
            tc.tile_pool(name="mat", bufs=5) as matp,
            tc.tile_pool(name="big32", bufs=7) as big32,
            tc.tile_pool(name="stage", bufs=4) as stagep,
            tc.tile_pool(name="psg", bufs=2, space="PSUM") as psg,
            tc.tile_pool(name="pstr", bufs=2, space="PSUM") as pstr,
            tc.tile_pool(name="dram", bufs=1, space="DRAM") as dramp,
        ):
            # ---- collective DRAM buffers --------------------------------
            cc1_in = dramp.tile([NCORES * D, UU + EE], BF16, name="cc1_in")
            cc1_out = dramp.tile([D, UU + EE], BF16, name="cc1_out")
            cc2_in = dramp.tile([NCORES * D, 3 * PP], BF16, name="cc2_in")
            cc2_out = dramp.tile([D, 3 * PP], BF16, name="cc2_out")

            # ---- constants ----------------------------------------------
            sb_w = constp.tile([D, 3, D], BF16, name="sb_w")
            nc.gpsimd.dma_start(sb_w[:], w3)
            sb_bT = constp.tile([D, 3], F32, name="sb_bT")
            nc.gpsimd.dma_start(sb_bT[:], bT3)
            sb_idf = constp.tile([D, D], F32, name="sb_idf")
            nc.gpsimd.dma_start(sb_idf[:], ident_f)
            sb_idb = constp.tile([D, D], BF16, name="sb_idb")
            nc.gpsimd.dma_start(sb_idb[:], ident_b)
            sb_peb = constp.tile([D, PP], BF16, name="sb_peb")
            nc.gpsimd.dma_start(sb_peb[:], peT_bf)
            sb_pef = constp.tile([D, PP], F32, name="sb_pef")
            nc.gpsimd.dma_start(sb_pef[:], peT_f)
            sel = constp.tile([128, KL, B], BF16, name="sel")
            nc.gpsimd.dma_start(sel[:], SelT[:].rearrange("(a p) n -> p a n", p=128))

            # gate natural tiles: fp8 (stream lhs) + bf16 (user-base lhs)
            nat8 = [
                constp.tile([128, KL, 128], FP8, name=f"nat8_{t}") for t in range(3)
            ]
            natb = {
                t: constp.tile([128, KL, 128], BF16, name=f"natb{t}") for t in (0, 2)
            }
            # own-block gates, transposed f32 (residual path)
            ownT = [
                big32.tile([D, PP], F32, tag="big32", name=f"ownT{t}")
                for t in range(3)
            ]

            # ---- gates (own block only) ---------------------------------
            for t in range(3):
                psz = psg.tile([D, PP], F32, tag="psg")
                for h in range(2):
                    cols = slice(512 * h, 512 * (h + 1))
                    nc.tensor.matmul(
                        psz[:, cols], sb_w[:, t, :], sb_peb[:, cols],
                        start=True, stop=True,
                    )
                sigT = stagep.tile([D, PP], F32, tag="stage")
                for h in range(2):
                    cols = slice(512 * h, 512 * (h + 1))
                    nc.scalar.activation(
                        sigT[:, cols], psz[:, cols], SIG, bias=sb_bT[:, t : t + 1]
                    )
                nc.vector.tensor_mul(ownT[t][:], sb_pef[:], sigT[:])
                for j in range(2):
                    pst = pstr.tile([128, 512], F32, tag="pstr")
                    for m in range(4):
                        c = (4 * j + m) * 128
                        nc.tensor.transpose(
                            pst[:, m * 128 : (m + 1) * 128],
                            ownT[t][:, c : c + 128],
                            sb_idf[:],
                        )
                    nc.scalar.activation(
                        nat8[t][:, 4 * j : 4 * j + 4, :], pst[:], COPY, scale=SX
                    )
                    if t in natb:
                        nc.vector.tensor_copy(
                            natb[t][:, 4 * j : 4 * j + 4, :], pst[:]
                        )

            # ---- user-base partials (early; frees PSUM via copy) --------
            def users_mm(ps, lhs_nat):
                for k in range(KL):
                    for h in range(2):
                        cols = slice(h * 512, h * 512 + 512)
                        nc.tensor.matmul(
                            ps[:, cols], lhs_nat[:, k, :], sel[:, k, cols],
                            start=(k == 0), stop=(k == KL - 1),
                        )

            users_base = constp.tile([D, 2 * B], F32, name="users_base")
            ps_bh = psg.tile([D, B], F32, tag="psg")
            users_mm(ps_bh, natb[0])
            nc.vector.tensor_copy(users_base[:, :B], ps_bh[:])

            # ---- contract-sharded partial stream ------------------------
            def partial_stream(matT, n_k, n_out, lhs, lhs_off, dst_fn, n_ch, ceng):
                """Stream matT (DRAM [n_k*128, n_out] fp8) against the
                natural gate tiles lhs[:, lhs_off + k, :] with DoubleRow
                pairs; psum out-chunks of 1024 cols are copied (scaled,
                bf16) to a stage tile and dst_fn(n, pay) DMAs each into
                the ReduceScatter payload rank-block layout."""
                kt_per = n_k // n_ch
                chunks = []
                for a in range(n_ch):
                    ch = matp.tile([128, kt_per, n_out], FP8, tag="mat")
                    eng = nc.sync if a % 2 == 0 else nc.scalar
                    eng.dma_start(
                        ch[:],
                        matT[a * kt_per * 128 : (a + 1) * kt_per * 128, :].rearrange(
                            "(a p) n -> p a n", p=128
                        ),
                    )
                    chunks.append(ch)
                for n in range(n_out // 1024):
                    ps = psg.tile([D, 1024], F32, tag="psg")
                    for j in range(n_k // 2):
                        a, jj = (2 * j) // kt_per, (2 * j) % kt_per
                        for h in range(2):
                            cols = slice(n * 1024 + h * 512, n * 1024 + h * 512 + 512)
                            nc.tensor.matmul(
                                ps[:, h * 512 : h * 512 + 512],
                                lhs[:, lhs_off + 2 * j : lhs_off + 2 * j + 2, :],
                                chunks[a][:, jj : jj + 2, cols],
                                start=(j == 0),
                                stop=(j == n_k // 2 - 1),
                                perf_mode=DR,
                            )
                    pay = stagep.tile([D, 1024], BF16, tag="stage")
                    if ceng is nc.vector:
                        ceng.tensor_scalar_mul(pay[:], ps[:], SAR)
                    else:
                        ceng.activation(pay[:], ps[:], COPY, scale=SAR)
                    dst_fn(n, pay)

            # RS1 payload: rank block r ([128, UU+EE] rows r*128..) holds
            # yu[:, r*512...] | yt[:, r*512...]; a [D, 1024] y-chunk n
            # spans rank blocks 2n and 2n+1.
            def rs1_dst(col0):
                def f(n, pay):
                    nc.gpsimd.dma_start(
                        cc1_in[2 * n * 128 : (2 * n + 2) * 128, col0 : col0 + 512]
                        .rearrange("(a p) c -> p a c", p=128),
                        pay[:].rearrange("p (a c) -> p a c", a=2),
                    )
                return f

            # RS2 payload: rank block r holds hg[:, r*1024...] |
            # tr[:, ...] | geo[:, ...]; chunk n IS rank n's slice.
            def rs2_dst(col0):
                def f(n, pay):
                    nc.gpsimd.dma_start(
                        cc2_in[n * 128 : (n + 1) * 128, col0 : col0 + 1024], pay[:]
                    )
                return f

            # ---- stage 1: y_up | y_tar partials + RS1 -------------------
            partial_stream(UpT, KL, U, nat8[0], 0, rs1_dst(0), 2, nc.vector)
            partial_stream(TarT, KL, E, nat8[1], 0, rs1_dst(512), 2, nc.vector)
            nc.gpsimd.collective_compute(
                "ReduceScatter", ADD, replica_groups=RG,
                ins=[cc1_in[:].opt()], outs=[cc1_out[:].opt()],
            )

            # geo-users base (after gate 2; overlaps stage-1 streams)
            ps_bg = psg.tile([D, B], F32, tag="psg")
            users_mm(ps_bg, natb[2])
            nc.vector.tensor_copy(users_base[:, B:], ps_bg[:])

            # ---- stage 2a: geo partial (no dependence on RS1) -----------
            partial_stream(GeoT, KL, P, nat8[2], 0, rs2_dst(2048), 4, nc.scalar)

            # ---- RS1 readback -> natural fp8 y tiles --------------------
            y_sb = stagep.tile([D, UU + EE], BF16, tag="stage", name="y_sb")
            nc.gpsimd.dma_start(y_sb[:], cc1_out[:])
            y_nat = constp.tile([128, KY, 128], FP8, name="y_nat")
            for j in range(2):
                pst = pstr.tile([128, 512], BF16, tag="pstr")
                for m in range(4):
                    c = (4 * j + m) * 128
                    nc.tensor.transpose(
                        pst[:, m * 128 : (m + 1) * 128],
                        y_sb[:, c : c + 128],
                        sb_idb[:],
                    )
                nc.vector.tensor_copy(y_nat[:, 4 * j : 4 * j + 4, :], pst[:])

            # ---- stage 2b: hg / trans delta partials + RS2 --------------
            partial_stream(PuT, UU // 128, P, y_nat, 0, rs2_dst(0), 2, nc.vector)
            partial_stream(
                SrcT, EE // 128, P, y_nat, UU // 128, rs2_dst(1024), 2, nc.vector
            )
            nc.gpsimd.collective_compute(
                "ReduceScatter", ADD, replica_groups=RG,
                ins=[cc2_in[:].opt()], outs=[cc2_out[:].opt()],
            )

            # ---- finalize pois ------------------------------------------
            d_sb = stagep.tile([D, 3 * PP], BF16, tag="stage", name="d_sb")
            nc.gpsimd.dma_start(d_sb[:], cc2_out[:])
            # segments: 0 = hg (gate x, slot 0), 1 = tr (gate s, slot 2),
            # 2 = geo (gate g, slot 1)
            for seg, t, slot, sc in ((0, 0, 0, S_DELTA), (1, 1, 2, S_DELTA),
                                     (2, 2, 1, S_GEO)):
                pT = big32.tile([D, PP], F32, tag="big32", name=f"pois{seg}")
                nc.vector.scalar_tensor_tensor(
                    pT[:], d_sb[:, seg * PP : (seg + 1) * PP], sc,
                    ownT[t][:], MULT, ADD,
                )
                nc.gpsimd.dma_start(poisT_o[slot], pT[:])

            # ---- user delta terms ---------------------------------------
            dn = {}
            for name, seg in (("hg", 0), ("geo", 2)):
                nat = constp.tile([128, KL, 128], BF16, name=f"dnat_{name}")
                for j in range(2):
                    pst = pstr.tile([128, 512], BF16, tag="pstr")
                    for m in range(4):
                        c = seg * PP + (4 * j + m) * 128
                        nc.tensor.transpose(
                            pst[:, m * 128 : (m + 1) * 128],
                            d_sb[:, c : c + 128],
                            sb_idb[:],
                        )
                    nc.vector.tensor_copy(nat[:, 4 * j : 4 * j + 4, :], pst[:])
                dn[name] = nat

            users_sb = constp.tile([D, 2 * B], F32, name="users_sb")
            for name, off, sc in (("hg", 0, S_DELTA), ("geo", B, S_GEO)):
                ps = psg.tile([D, B], F32, tag="psg")
                users_mm(ps, dn[name])
                nc.vector.scalar_tensor_tensor(
                    users_sb[:, off : off + B], ps[:], sc,
                    users_base[:, off : off + B], MULT, ADD,
                )
            nc.gpsimd.dma_start(usersT_o, users_sb[:])

    nc.compile()
    return nc


def _get_nc():
    if "nc" not in _CACHE:
        _CACHE["nc"] = _build_nc()
    return _CACHE["nc"]


def _shard_inputs(inputs):
    f32 = np.float32
    bf16 = ml_dtypes.bfloat16
    fp8 = ml_dtypes.float8_e4m3
    pe = np.asarray(inputs["poi_emb_weight"], f32)[:P]
    peT = np.ascontiguousarray(pe.T)                     # [D, P]
    w3 = np.stack(
        [
            np.asarray(inputs["w_gate_col"], f32),
            np.asarray(inputs["w_gate_seq"], f32),
            np.asarray(inputs["w_gate_geo"], f32),
        ],
        axis=1,
    ).astype(bf16)                                        # [D, 3, D]
    bT3 = np.stack(
        [
            np.asarray(inputs["b_gate_col"], f32)[0],
            np.asarray(inputs["b_gate_seq"], f32)[0],
            np.asarray(inputs["b_gate_geo"], f32)[0],
        ],
        axis=1,
    )
    eye = np.eye(D, dtype=f32)
    idx = np.asarray(inputs["user_idx"]).astype(np.int64)
    Up = np.asarray(inputs["HG_up"], f32)                 # [U, P]
    Pu = np.asarray(inputs["HG_pu"], f32)                 # [P, U]
    Tar = np.asarray(inputs["HG_poi_tar"], f32)           # [E, P]
    Src = np.asarray(inputs["HG_poi_src"], f32)           # [P, E]
    Geo = np.asarray(inputs["poi_geo_graph"], f32)        # [P, P]
    Sel = Up[idx]                                         # [B, P]
    S18, S17 = 2.0 ** 18, 2.0 ** 17

    in_maps = []
    for i in range(NCORES):
        rp = slice(PP * i, PP * (i + 1))
        ru = slice(UU * i, UU * (i + 1))
        re_ = slice(EE * i, EE * (i + 1))
        in_maps.append(
            {
                "peT_bf": peT[:, rp].astype(bf16),
                "peT_f": np.ascontiguousarray(peT[:, rp]),
                "w3": w3,
                "bT3": bT3,
                "ident_f": eye,
                "ident_b": eye.astype(bf16),
                "UpT": (Up.T[rp] * S18).astype(fp8),
                "TarT": (Tar.T[rp] * S18).astype(fp8),
                "GeoT": (Geo.T[rp] * S18).astype(fp8),
                "PuT": (Pu[:, ru].T * S17).astype(fp8),
                "SrcT": (Src[:, re_].T * S17).astype(fp8),
                "SelT": np.ascontiguousarray(Sel[:, rp].T).astype(bf16),
            }
        )
    return in_maps


def _assemble(results):
    f32 = np.float32
    hg = np.empty((P, D), f32)
    geo = np.empty((P, D), f32)
    tr = np.empty((P, D), f32)
    users = np.zeros((D, 2 * B), f32)
    for i in range(NCORES):
        rp = slice(PP * i, PP * (i + 1))
        pois = results[i]["poisT_o"]
        hg[rp] = pois[0].T
        geo[rp] = pois[1].T
        tr[rp] = pois[2].T
        users += results[i]["usersT_o"]
    return np.concatenate([hg, geo, tr, users[:, :B].T, users[:, B:].T], axis=0)


def _run(inputs, trace=False, **spmd_kwargs):
    nc = _get_nc()
    in_maps = _shard_inputs(inputs)
    res = run_bass_kernel_spmd(
        nc, in_maps, list(range(NCORES)), trace=trace, **spmd_kwargs
    )
    return _assemble(res.results), res


def kernel(**inputs):
    return _run(inputs)[0]


if __name__ == "__main__":
    import pickle

    with open("/tmp/inputs.pkl", "rb") as f:
        inputs = pickle.load(f)
    out = kernel(**inputs)
    exp = np.load("/tmp/expected.npy")
    rel = np.linalg.norm(out - exp) / np.linalg.norm(exp)
    print("Relative error:", rel)


# revision 17
# speedup vs baseline: 1.3462x; 1.3462x over previous
"""Trainium2 Bass kernel for nn_HODE_MDP (hypergraph ODE message passing).

Math (T_UP = T_GEO = T_P2P = 1.0, ALPHA = 0.8):
    pe  = poi_emb_weight[:-1]                      # [P, D]
    x/s/g = pe * sigmoid(pe @ W_t + b_t)           # col / seq / geo gates
    hg_pois    = x + HG_pu @ (HG_up @ x)
    geo_pois   = g + 0.4 * (poi_geo_graph @ g)
    trans_pois = s + HG_poi_src @ (HG_poi_tar @ s)
    hg_users   = (HG_up @ hg_pois)[user_idx]
    geo_users  = (HG_up @ geo_pois)[user_idx]
    out = concat([hg_pois, geo_pois, trans_pois, hg_users, geo_users])

Distribution (8 NeuronCores), v2 — contract-dim sharding upstream:
  * y_up = HG_up@x, y_tar = Tar@s, Geo@g are sharded over the CONTRACT
    dim (P): each core uses only its LOCAL gate block (gates computed
    for the own 1024-row block only) against the matching column block
    of each matrix, producing full-width partials.  One fused AllReduce
    ([D, U+E] bf16) combines y_up|y_tar; a ReduceScatter combines the
    geo partial straight into each core's own row-slice.
  * hg/trans deltas row-shard over P (full y_up/y_tar stationary after
    the AllReduce).
  * user embeddings: host pre-gathers Up[user_idx] -> [B, P]; each core
    contracts its local P-block of that against its local hg/geo pois
    blocks, and the HOST sums the 8 partial [D, 2B] outputs (no
    device collective for users at all).

All big streams are fp8 e4m3 with power-of-two pre-scaling (host side)
and DoubleRow matmuls (2 k-tiles per instruction = 2x PE throughput).
The delta terms are ~3e-3 of the residual scale, so fp8 on the delta
paths costs ~6e-5 relative error; the user matmul runs in bf16.
Measured end-to-end ~8e-5.
"""

import sys

if "/opt/trn_rl_repo" not in sys.path:
    sys.path.insert(0, "/opt/trn_rl_repo")

import numpy as np
import ml_dtypes

import concourse.bass as bass  # noqa: F401
import concourse.bacc as bacc
import concourse.mybir as mybir
import concourse.tile as tile
from concourse.bass_utils import run_bass_kernel_spmd

F32 = mybir.dt.float32
BF16 = mybir.dt.bfloat16
FP8 = mybir.dt.float8e4
SIG = mybir.ActivationFunctionType.Sigmoid
COPY = mybir.ActivationFunctionType.Copy
MULT = mybir.AluOpType.mult
ADD = mybir.AluOpType.add
DR = mybir.MatmulPerfMode.DoubleRow

NCORES = 8
P, U, E, D, B = 8192, 4096, 4096, 128, 1024
PP = P // NCORES            # 1024 rows per core
KL = PP // 128              # 8 local k-tiles
RG = [list(range(NCORES))]

SX = 128.0                  # gate -> fp8 scale (2^7)
SAR = 2.0 ** -7             # psum -> AllReduce payload scale
SHG = 2.0 ** -35            # C-phase psum -> f32 delta scale
GEO_SCALE = 0.4 * 2.0 ** -18  # geo payload (2^18 * Geo@g) -> delta

_CACHE: dict = {}


def _build_nc():
    nc = bacc.Bacc(
        "TRN2",
        target_bir_lowering=False,
        debug=False,
        enable_asserts=False,
        num_devices=NCORES,
    )

    # ---- per-core DRAM I/O ----------------------------------------------
    peT_bf = nc.dram_tensor("peT_bf", [D, PP], BF16, kind="ExternalInput").ap()
    peT_f = nc.dram_tensor("peT_f", [D, PP], F32, kind="ExternalInput").ap()
    w3 = nc.dram_tensor("w3", [D, 3, D], BF16, kind="ExternalInput").ap()
    bT3 = nc.dram_tensor("bT3", [D, 3], F32, kind="ExternalInput").ap()
    ident_f = nc.dram_tensor("ident_f", [D, D], F32, kind="ExternalInput").ap()
    ident_b = nc.dram_tensor("ident_b", [D, D], BF16, kind="ExternalInput").ap()
    UpT = nc.dram_tensor("UpT", [PP, U], FP8, kind="ExternalInput").ap()
    TarT = nc.dram_tensor("TarT", [PP, E], FP8, kind="ExternalInput").ap()
    GeoT = nc.dram_tensor("GeoT", [PP, P], FP8, kind="ExternalInput").ap()
    PuT = nc.dram_tensor("PuT", [U, PP], FP8, kind="ExternalInput").ap()
    SrcT = nc.dram_tensor("SrcT", [E, PP], FP8, kind="ExternalInput").ap()
    SelT = nc.dram_tensor("SelT", [PP, B], BF16, kind="ExternalInput").ap()

    poisT_o = nc.dram_tensor("poisT_o", [3, D, PP], F32, kind="ExternalOutput").ap()
    usersT_o = nc.dram_tensor("usersT_o", [D, 2 * B], F32, kind="ExternalOutput").ap()

    with tile.TileContext(nc) as tc:
        with (
            tc.tile_pool(name="const", bufs=1) as constp,
            tc.tile_pool(name="mat", bufs=4) as matp,
            tc.tile_pool(name="big32", bufs=6) as big32,
            tc.tile_pool(name="stage", bufs=3) as stagep,
            tc.tile_pool(name="outp", bufs=1) as outp,
            tc.tile_pool(name="psg", bufs=2, space="PSUM") as psg,
            tc.tile_pool(name="pstr", bufs=2, space="PSUM") as pstr,
            tc.tile_pool(name="dram", bufs=1, space="DRAM") as dramp,
        ):
            # ---- collective DRAM buffers --------------------------------
            cc_ar_in = dramp.tile([D, U + E], FP8, name="cc_ar_in")
            cc_ar_out = dramp.tile(
                [D, U + E], FP8, addr_space="Shared", name="cc_ar_out"
            )
            cc_rs_in = dramp.tile([NCORES * D, PP], FP8, name="cc_rs_in")
            cc_rs_out = dramp.tile([D, PP], FP8, name="cc_rs_out")

            # ---- constants ----------------------------------------------
            sb_w = constp.tile([D, 3, D], BF16, name="sb_w")
            nc.gpsimd.dma_start(sb_w[:], w3)
            sb_bT = constp.tile([D, 3], F32, name="sb_bT")
            nc.gpsimd.dma_start(sb_bT[:], bT3)
            sb_idf = constp.tile([D, D], F32, name="sb_idf")
            nc.gpsimd.dma_start(sb_idf[:], ident_f)
            sb_idb = constp.tile([D, D], BF16, name="sb_idb")
            nc.gpsimd.dma_start(sb_idb[:], ident_b)
            sb_peb = constp.tile([D, PP], BF16, name="sb_peb")
            nc.gpsimd.dma_start(sb_peb[:], peT_bf)
            sb_pef = constp.tile([D, PP], F32, name="sb_pef")
            nc.gpsimd.dma_start(sb_pef[:], peT_f)
            sel = constp.tile([128, KL, B], BF16, name="sel")
            nc.gpsimd.dma_start(sel[:], SelT[:].rearrange("(a p) n -> p a n", p=128))

            # gate natural fp8 tiles (stationary lhs for streams)
            nat = [
                constp.tile([128, KL, 128], FP8, name=f"nat{t}") for t in range(3)
            ]
            # own-block gates, transposed f32 (residual path)
            ownT = [
                big32.tile([D, PP], F32, tag="big32", name=f"ownT{t}")
                for t in range(3)
            ]

            # ---- gates (own block only) ---------------------------------
            for t in range(3):
                psz = psg.tile([D, PP], F32, tag="psg")
                for h in range(2):
                    cols = slice(512 * h, 512 * (h + 1))
                    nc.tensor.matmul(
                        psz[:, cols], sb_w[:, t, :], sb_peb[:, cols],
                        start=True, stop=True,
                    )
                sigT = stagep.tile([D, PP], F32, tag="sig")
                for h in range(2):
                    cols = slice(512 * h, 512 * (h + 1))
                    nc.scalar.activation(
                        sigT[:, cols], psz[:, cols], SIG, bias=sb_bT[:, t : t + 1]
                    )
                nc.vector.tensor_mul(ownT[t][:], sb_pef[:], sigT[:])
                for j in range(2):
                    pst = pstr.tile([128, 512], F32, tag="pstr")
                    for m in range(4):
                        c = (4 * j + m) * 128
                        nc.tensor.transpose(
                            pst[:, m * 128 : (m + 1) * 128],
                            ownT[t][:, c : c + 128],
                            sb_idf[:],
                        )
                    nc.scalar.activation(
                        nat[t][:, 4 * j : 4 * j + 4, :], pst[:], COPY, scale=SX
                    )

            # ---- stream helper: contract-sharded partial ----------------
            def partial_stream(matT, n_out, lhs, pay, pay_off, n_tiles, vec_copy):
                kt_per = KL // n_tiles
                chunks = []
                for a in range(n_tiles):
                    ch = matp.tile([128, kt_per, n_out], FP8, tag="mat")
                    eng = nc.sync if a % 2 == 0 else nc.scalar
                    eng.dma_start(
                        ch[:],
                        matT[a * kt_per * 128 : (a + 1) * kt_per * 128, :].rearrange(
                            "(a p) n -> p a n", p=128
                        ),
                    )
                    chunks.append(ch)
                for n in range(n_out // 1024):
                    ps = psg.tile([D, 1024], F32, tag="psg")
                    for j in range(KL // 2):
                        a, jj = (2 * j) // kt_per, (2 * j) % kt_per
                        for h in range(2):
                            cols = slice(n * 1024 + h * 512, n * 1024 + h * 512 + 512)
                            nc.tensor.matmul(
                                ps[:, h * 512 : h * 512 + 512],
                                lhs[:, 2 * j : 2 * j + 2, :],
                                chunks[a][:, jj : jj + 2, cols],
                                start=(j == 0),
                                stop=(j == KL // 2 - 1),
                                perf_mode=DR,
                            )
                    if vec_copy:
                        # bf16 chunk -> PE transpose -> natural fp8 payload
                        tmp = stagep.tile([D, 1024], BF16, tag="sig")
                        nc.vector.tensor_scalar_mul(tmp[:], ps[:], SAR)
                        nat8 = stagep.tile([128, 8, 128], FP8, tag="nat8")
                        for j2 in range(2):
                            pst = pstr.tile([128, 512], BF16, tag="pstr")
                            for m in range(4):
                                c = (4 * j2 + m) * 128
                                nc.tensor.transpose(
                                    pst[:, m * 128 : (m + 1) * 128],
                                    tmp[:, c : c + 128],
                                    sb_idb[:],
                                )
                            if j2 == 0:
                                nc.vector.tensor_copy(
                                    nat8[:, :4, :], pst[:]
                                )
                            else:
                                nc.scalar.activation(
                                    nat8[:, 4:, :], pst[:], COPY
                                )
                        nc.gpsimd.dma_start(
                            pay[:, pay_off + n * 1024 : pay_off + (n + 1) * 1024],
                            nat8[:].rearrange("p a f -> p (a f)"),
                        )
                    else:
                        dst = pay[:, pay_off + n * 1024 : pay_off + (n + 1) * 1024]
                        nc.scalar.activation(dst, ps[:], COPY, scale=SAR)

            # ---- B1/B2: y_up | y_tar partials + fused AllReduce ---------
            # payload is already NATURAL layout fp8 (transposed pre-AR)
            partial_stream(UpT, U, nat[0], cc_ar_in, 0, 2, True)
            partial_stream(TarT, E, nat[1], cc_ar_in, U, 2, True)
            nc.gpsimd.collective_compute(
                "AllReduce",
                ADD,
                replica_groups=RG,
                ins=[cc_ar_in[:].opt()],
                outs=[cc_ar_out[:].opt()],
            )

            # ---- B3: geo partial + ReduceScatter ------------------------
            geo_part = constp.tile([D, P], FP8, name="geo_part")
            partial_stream(GeoT, P, nat[2], geo_part, 0, 4, False)
            nc.gpsimd.dma_start(
                cc_rs_in[:].rearrange("(r p) c -> p r c", p=128),
                geo_part[:].rearrange("p (r c) -> p r c", r=NCORES),
            )
            nc.gpsimd.collective_compute(
                "ReduceScatter",
                ADD,
                replica_groups=RG,
                ins=[cc_rs_in[:].opt()],
                outs=[cc_rs_out[:].opt()],
            )

            # ---- AllReduce readback: payload IS natural layout ----------
            yu_nat = constp.tile([128, U // 128, 128], FP8, name="yu_nat")
            yt_nat = constp.tile([128, E // 128, 128], FP8, name="yt_nat")
            nc.gpsimd.dma_start(
                yu_nat[:].rearrange("p a f -> p (a f)"), cc_ar_out[:, :U]
            )
            nc.gpsimd.dma_start(
                yt_nat[:].rearrange("p a f -> p (a f)"), cc_ar_out[:, U:]
            )

            # ---- row-sharded delta stream (C1 / C2) ---------------------
            def delta_stream(matT, n_k, lhs):
                kt_per = n_k // 2
                chunks = []
                for a in range(2):
                    ch = matp.tile([128, kt_per, PP], FP8, tag="mat")
                    eng = nc.sync if a % 2 == 0 else nc.scalar
                    eng.dma_start(
                        ch[:],
                        matT[a * kt_per * 128 : (a + 1) * kt_per * 128, :].rearrange(
                            "(a p) n -> p a n", p=128
                        ),
                    )
                    chunks.append(ch)
                ps = psg.tile([D, PP], F32, tag="psg")
                for j in range(n_k // 2):
                    a, jj = (2 * j) // kt_per, (2 * j) % kt_per
                    for h in range(2):
                        nc.tensor.matmul(
                            ps[:, h * 512 : h * 512 + 512],
                            lhs[:, 2 * j : 2 * j + 2, :],
                            chunks[a][:, jj : jj + 2, h * 512 : h * 512 + 512],
                            start=(j == 0),
                            stop=(j == n_k // 2 - 1),
                            perf_mode=DR,
                        )
                return ps

            def make_nat(srcT, dst):
                for j in range(2):
                    pst = pstr.tile([128, 512], F32, tag="pstr")
                    for m in range(4):
                        c = (4 * j + m) * 128
                        nc.tensor.transpose(
                            pst[:, m * 128 : (m + 1) * 128],
                            srcT[:, c : c + 128],
                            sb_idf[:],
                        )
                    if j == 0:
                        nc.vector.tensor_copy(dst[:, 4 * j : 4 * j + 4, :], pst[:])
                    else:
                        nc.scalar.activation(
                            dst[:, 4 * j : 4 * j + 4, :], pst[:], COPY
                        )

            # C1: hg_pois = x + Pu @ y_up
            ps_hg = delta_stream(PuT, U // 128, yu_nat)
            hg_poisT = big32.tile([D, PP], F32, tag="big32", name="hg_poisT")
            nc.vector.scalar_tensor_tensor(
                hg_poisT[:], ps_hg[:], SHG, ownT[0][:], MULT, ADD
            )
            nc.gpsimd.dma_start(poisT_o[0], hg_poisT[:])
            hg_nat = constp.tile([128, KL, 128], BF16, name="hg_nat")
            make_nat(hg_poisT, hg_nat)

            # C2: trans_pois = s + Src @ y_tar
            ps_tr = delta_stream(SrcT, E // 128, yt_nat)
            trans_poisT = big32.tile([D, PP], F32, tag="big32", name="trans_poisT")
            nc.vector.scalar_tensor_tensor(
                trans_poisT[:], ps_tr[:], SHG, ownT[1][:], MULT, ADD
            )
            nc.sync.dma_start(poisT_o[2], trans_poisT[:])

            # ---- geo finalize (ReduceScatter output) --------------------
            geo_sum = stagep.tile([D, PP], FP8, tag="sig", name="geo_sum")
            nc.gpsimd.dma_start(geo_sum[:], cc_rs_out[:])
            geo_poisT = big32.tile([D, PP], F32, tag="big32", name="geo_poisT")
            nc.vector.scalar_tensor_tensor(
                geo_poisT[:], geo_sum[:], GEO_SCALE, ownT[2][:], MULT, ADD
            )
            nc.scalar.dma_start(poisT_o[1], geo_poisT[:])
            geo_nat = constp.tile([128, KL, 128], BF16, name="geo_nat")
            make_nat(geo_poisT, geo_nat)

            # ---- D: user partials (bf16, host reduces) ------------------
            ps_hu = psg.tile([D, B], F32, tag="psg")
            ps_gu = psg.tile([D, B], F32, tag="psg")
            for k in range(KL):
                for h in range(2):
                    cols = slice(h * 512, h * 512 + 512)
                    nc.tensor.matmul(
                        ps_hu[:, cols], hg_nat[:, k, :], sel[:, k, cols],
                        start=(k == 0), stop=(k == KL - 1),
                    )
                    nc.tensor.matmul(
                        ps_gu[:, cols], geo_nat[:, k, :], sel[:, k, cols],
                        start=(k == 0), stop=(k == KL - 1),
                    )
            users_sb = outp.tile([D, 2 * B], F32, name="users_sb")
            nc.vector.tensor_copy(users_sb[:, :B], ps_hu[:])
            nc.scalar.activation(users_sb[:, B:], ps_gu[:], COPY)
            nc.sync.dma_start(usersT_o[:, :B], users_sb[:, :B])
            nc.scalar.dma_start(usersT_o[:, B:], users_sb[:, B:])

    nc.compile()
    return nc


def _get_nc():
    if "nc" not in _CACHE:
        _CACHE["nc"] = _build_nc()
    return _CACHE["nc"]


def _shard_inputs(inputs):
    f32 = np.float32
    bf16 = ml_dtypes.bfloat16
    fp8 = ml_dtypes.float8_e4m3
    pe = np.asarray(inputs["poi_emb_weight"], f32)[:P]
    peT = np.ascontiguousarray(pe.T)                     # [D, P]
    w3 = np.stack(
        [
            np.asarray(inputs["w_gate_col"], f32),
            np.asarray(inputs["w_gate_seq"], f32),
            np.asarray(inputs["w_gate_geo"], f32),
        ],
        axis=1,
    ).astype(bf16)                                        # [D, 3, D]
    bT3 = np.stack(
        [
            np.asarray(inputs["b_gate_col"], f32)[0],
            np.asarray(inputs["b_gate_seq"], f32)[0],
            np.asarray(inputs["b_gate_geo"], f32)[0],
        ],
        axis=1,
    )
    eye = np.eye(D, dtype=f32)
    idx = np.asarray(inputs["user_idx"]).astype(np.int64)
    Up = np.asarray(inputs["HG_up"], f32)                 # [U, P]
    Pu = np.asarray(inputs["HG_pu"], f32)                 # [P, U]
    Tar = np.asarray(inputs["HG_poi_tar"], f32)           # [E, P]
    Src = np.asarray(inputs["HG_poi_src"], f32)           # [P, E]
    Geo = np.asarray(inputs["poi_geo_graph"], f32)        # [P, P]
    Sel = Up[idx]                                         # [B, P]
    S18, S17 = 2.0 ** 18, 2.0 ** 17

    in_maps = []
    for i in range(NCORES):
        rp = slice(PP * i, PP * (i + 1))
        in_maps.append(
            {
                "peT_bf": peT[:, rp].astype(bf16),
                "peT_f": np.ascontiguousarray(peT[:, rp]),
                "w3": w3,
                "bT3": bT3,
                "ident_f": eye,
                "ident_b": eye.astype(bf16),
                "UpT": (Up.T[rp] * S18).astype(fp8),
                "TarT": (Tar.T[rp] * S18).astype(fp8),
                "GeoT": (Geo.T[rp] * S18).astype(fp8),
                "PuT": (Pu[rp].T * S17).astype(fp8),
                "SrcT": (Src[rp].T * S17).astype(fp8),
                "SelT": np.ascontiguousarray(Sel[:, rp].T).astype(bf16),
            }
        )
    return in_maps


def _assemble(results):
    f32 = np.float32
    hg = np.empty((P, D), f32)
    geo = np.empty((P, D), f32)
    tr = np.empty((P, D), f32)
    users = np.zeros((D, 2 * B), f32)
    for i in range(NCORES):
        rp = slice(PP * i, PP * (i + 1))
        pois = results[i]["poisT_o"]
        hg[rp] = pois[0].T
        geo[rp] = pois[1].T
        tr[rp] = pois[2].T
        users += results[i]["usersT_o"]
    return np.concatenate([hg, geo, tr, users[:, :B].T, users[:, B:].T], axis=0)


def _run(inputs, trace=False, **spmd_kwargs):
    nc = _get_nc()
    in_maps = _shard_inputs(inputs)
    res = run_bass_kernel_spmd(
        nc, in_maps, list(range(NCORES)), trace=trace, **spmd_kwargs
    )
    return _assemble(res.results), res


def kernel(**inputs):
    return _run(inputs)[0]


if __name__ == "__main__":
    import pickle

    with open("/tmp/inputs.pkl", "rb") as f:
        inputs = pickle.load(f)
    out = kernel(**inputs)
    exp = np.load("/tmp/expected.npy")
    rel = np.linalg.norm(out - exp) / np.linalg.norm(exp)
    print("Relative error:", rel)


# revision 18
# speedup vs baseline: 1.4188x; 1.0539x over previous
"""Trainium2 Bass kernel for nn_HODE_MDP (hypergraph ODE message passing).

Math (T_UP = T_GEO = T_P2P = 1.0, ALPHA = 0.8):
    pe  = poi_emb_weight[:-1]                      # [P, D]
    x/s/g = pe * sigmoid(pe @ W_t + b_t)           # col / seq / geo gates
    hg_pois    = x + HG_pu @ (HG_up @ x)
    geo_pois   = g + 0.4 * (poi_geo_graph @ g)
    trans_pois = s + HG_poi_src @ (HG_poi_tar @ s)
    hg_users   = (HG_up @ hg_pois)[user_idx]
    geo_users  = (HG_up @ geo_pois)[user_idx]
    out = concat([hg_pois, geo_pois, trans_pois, hg_users, geo_users])

Distribution (8 NeuronCores), v2 — contract-dim sharding upstream:
  * y_up = HG_up@x, y_tar = Tar@s, Geo@g are sharded over the CONTRACT
    dim (P): each core uses only its LOCAL gate block (gates computed
    for the own 1024-row block only) against the matching column block
    of each matrix, producing full-width partials.  One fused AllReduce
    ([D, U+E] bf16) combines y_up|y_tar; a ReduceScatter combines the
    geo partial straight into each core's own row-slice.
  * hg/trans deltas row-shard over P (full y_up/y_tar stationary after
    the AllReduce).
  * user embeddings: host pre-gathers Up[user_idx] -> [B, P]; each core
    contracts its local P-block of that against its local hg/geo pois
    blocks, and the HOST sums the 8 partial [D, 2B] outputs (no
    device collective for users at all).

All big streams are fp8 e4m3 with power-of-two pre-scaling (host side)
and DoubleRow matmuls (2 k-tiles per instruction = 2x PE throughput).
The delta terms are ~3e-3 of the residual scale, so fp8 on the delta
paths costs ~6e-5 relative error; the user matmul runs in bf16.
Measured end-to-end ~8e-5.
"""

import sys

if "/opt/trn_rl_repo" not in sys.path:
    sys.path.insert(0, "/opt/trn_rl_repo")

import numpy as np
import ml_dtypes

import concourse.bass as bass  # noqa: F401
import concourse.bacc as bacc
import concourse.mybir as mybir
import concourse.tile as tile
from concourse.bass_utils import run_bass_kernel_spmd

F32 = mybir.dt.float32
BF16 = mybir.dt.bfloat16
FP8 = mybir.dt.float8e4
SIG = mybir.ActivationFunctionType.Sigmoid
COPY = mybir.ActivationFunctionType.Copy
MULT = mybir.AluOpType.mult
ADD = mybir.AluOpType.add
DR = mybir.MatmulPerfMode.DoubleRow

NCORES = 8
P, U, E, D, B = 8192, 4096, 4096, 128, 1024
PP = P // NCORES            # 1024 rows per core
KL = PP // 128              # 8 local k-tiles
RG = [list(range(NCORES))]

SX = 128.0                  # gate -> fp8 scale (2^7)
SAR = 2.0 ** -7             # psum -> AllReduce payload scale
SHG = 2.0 ** -35            # C-phase psum -> f32 delta scale
GEO_SCALE = 0.4 * 2.0 ** -18  # geo payload (2^18 * Geo@g) -> delta

_CACHE: dict = {}


def _build_nc():
    nc = bacc.Bacc(
        "TRN2",
        target_bir_lowering=False,
        debug=False,
        enable_asserts=False,
        num_devices=NCORES,
    )

    # ---- per-core DRAM I/O ----------------------------------------------
    peT_bf = nc.dram_tensor("peT_bf", [D, PP], BF16, kind="ExternalInput").ap()
    peT_f = nc.dram_tensor("peT_f", [D, PP], F32, kind="ExternalInput").ap()
    w3 = nc.dram_tensor("w3", [D, 3, D], BF16, kind="ExternalInput").ap()
    bT3 = nc.dram_tensor("bT3", [D, 3], F32, kind="ExternalInput").ap()
    ident_f = nc.dram_tensor("ident_f", [D, D], F32, kind="ExternalInput").ap()
    ident_b = nc.dram_tensor("ident_b", [D, D], BF16, kind="ExternalInput").ap()
    UpT = nc.dram_tensor("UpT", [PP, U], FP8, kind="ExternalInput").ap()
    TarT = nc.dram_tensor("TarT", [PP, E], FP8, kind="ExternalInput").ap()
    GeoT = nc.dram_tensor("GeoT", [PP, P], FP8, kind="ExternalInput").ap()
    PuT = nc.dram_tensor("PuT", [U, PP], FP8, kind="ExternalInput").ap()
    SrcT = nc.dram_tensor("SrcT", [E, PP], FP8, kind="ExternalInput").ap()
    SelT = nc.dram_tensor("SelT", [PP, B], BF16, kind="ExternalInput").ap()

    poisT_o = nc.dram_tensor("poisT_o", [3, D, PP], F32, kind="ExternalOutput").ap()
    usersT_o = nc.dram_tensor("usersT_o", [D, 2 * B], F32, kind="ExternalOutput").ap()

    with tile.TileContext(nc) as tc:
        with (
            tc.tile_pool(name="const", bufs=1) as constp,
            tc.tile_pool(name="mat", bufs=4) as matp,
            tc.tile_pool(name="big32", bufs=6) as big32,
            tc.tile_pool(name="stage", bufs=3) as stagep,
            tc.tile_pool(name="outp", bufs=1) as outp,
            tc.tile_pool(name="psg", bufs=2, space="PSUM") as psg,
            tc.tile_pool(name="pstr", bufs=2, space="PSUM") as pstr,
            tc.tile_pool(name="dram", bufs=1, space="DRAM") as dramp,
        ):
            # ---- collective DRAM buffers --------------------------------
            cc_ar_in = dramp.tile([D, U + E], FP8, name="cc_ar_in")
            cc_ar_out = dramp.tile(
                [D, U + E], FP8, addr_space="Shared", name="cc_ar_out"
            )
            cc_rs_in = dramp.tile([NCORES * D, PP], FP8, name="cc_rs_in")
            cc_rs_out = dramp.tile([D, PP], FP8, name="cc_rs_out")

            # ---- constants ----------------------------------------------
            sb_w = constp.tile([D, 3, D], BF16, name="sb_w")
            nc.gpsimd.dma_start(sb_w[:], w3)
            sb_bT = constp.tile([D, 3], F32, name="sb_bT")
            nc.gpsimd.dma_start(sb_bT[:], bT3)
            sb_idf = constp.tile([D, D], F32, name="sb_idf")
            nc.gpsimd.dma_start(sb_idf[:], ident_f)
            sb_idb = constp.tile([D, D], BF16, name="sb_idb")
            nc.gpsimd.dma_start(sb_idb[:], ident_b)
            sb_peb = constp.tile([D, PP], BF16, name="sb_peb")
            nc.gpsimd.dma_start(sb_peb[:], peT_bf)
            sb_pef = constp.tile([D, PP], F32, name="sb_pef")
            nc.gpsimd.dma_start(sb_pef[:], peT_f)
            sel = constp.tile([128, KL, B], BF16, name="sel")
            nc.gpsimd.dma_start(sel[:], SelT[:].rearrange("(a p) n -> p a n", p=128))

            # gate natural fp8 tiles (stationary lhs for streams)
            nat = [
                constp.tile([128, KL, 128], FP8, name=f"nat{t}") for t in range(3)
            ]
            # own-block gates, transposed f32 (residual path)
            ownT = [
                big32.tile([D, PP], F32, tag="big32", name=f"ownT{t}")
                for t in range(3)
            ]

            # ---- gates (own block only) ---------------------------------
            for t in range(3):
                psz = psg.tile([D, PP], F32, tag="psg")
                for h in range(2):
                    cols = slice(512 * h, 512 * (h + 1))
                    nc.tensor.matmul(
                        psz[:, cols], sb_w[:, t, :], sb_peb[:, cols],
                        start=True, stop=True,
                    )
                sigT = stagep.tile([D, PP], F32, tag="sig")
                for h in range(2):
                    cols = slice(512 * h, 512 * (h + 1))
                    nc.scalar.activation(
                        sigT[:, cols], psz[:, cols], SIG, bias=sb_bT[:, t : t + 1]
                    )
                nc.vector.tensor_mul(ownT[t][:], sb_pef[:], sigT[:])
                for j in range(2):
                    pst = pstr.tile([128, 512], F32, tag="pstr")
                    for m in range(4):
                        c = (4 * j + m) * 128
                        nc.tensor.transpose(
                            pst[:, m * 128 : (m + 1) * 128],
                            ownT[t][:, c : c + 128],
                            sb_idf[:],
                        )
                    nc.scalar.activation(
                        nat[t][:, 4 * j : 4 * j + 4, :], pst[:], COPY, scale=SX
                    )

            # ---- stream helper: contract-sharded partial ----------------
            def partial_stream(matT, n_out, lhs, pay, pay_off, n_tiles, vec_copy):
                kt_per = KL // n_tiles
                chunks = []
                for a in range(n_tiles):
                    ch = matp.tile([128, kt_per, n_out], FP8, tag="mat")
                    eng = nc.sync
                    eng.dma_start(
                        ch[:],
                        matT[a * kt_per * 128 : (a + 1) * kt_per * 128, :].rearrange(
                            "(a p) n -> p a n", p=128
                        ),
                    )
                    chunks.append(ch)
                for n in range(n_out // 1024):
                    ps = psg.tile([D, 1024], F32, tag="psg")
                    for j in range(KL // 2):
                        a, jj = (2 * j) // kt_per, (2 * j) % kt_per
                        for h in range(2):
                            cols = slice(n * 1024 + h * 512, n * 1024 + h * 512 + 512)
                            nc.tensor.matmul(
                                ps[:, h * 512 : h * 512 + 512],
                                lhs[:, 2 * j : 2 * j + 2, :],
                                chunks[a][:, jj : jj + 2, cols],
                                start=(j == 0),
                                stop=(j == KL // 2 - 1),
                                perf_mode=DR,
                            )
                    if vec_copy:
                        # bf16 chunk -> PE transpose -> natural fp8 payload
                        tmp = stagep.tile([D, 1024], BF16, tag="sig")
                        nc.vector.tensor_scalar_mul(tmp[:], ps[:], SAR)
                        nat8 = stagep.tile([128, 8, 128], FP8, tag="nat8")
                        for j2 in range(2):
                            pst = pstr.tile([128, 512], BF16, tag="pstr")
                            for m in range(4):
                                c = (4 * j2 + m) * 128
                                nc.tensor.transpose(
                                    pst[:, m * 128 : (m + 1) * 128],
                                    tmp[:, c : c + 128],
                                    sb_idb[:],
                                )
                            nc.vector.tensor_copy(
                                nat8[:, 4 * j2 : 4 * j2 + 4, :], pst[:]
                            )
                        nc.gpsimd.dma_start(
                            pay[:, pay_off + n * 1024 : pay_off + (n + 1) * 1024],
                            nat8[:].rearrange("p a f -> p (a f)"),
                        )
                    else:
                        dst = pay[:, pay_off + n * 1024 : pay_off + (n + 1) * 1024]
                        nc.scalar.activation(dst, ps[:], COPY, scale=SAR)

            # ---- B1/B2: y_up | y_tar partials + fused AllReduce ---------
            # payload is already NATURAL layout fp8 (transposed pre-AR)
            partial_stream(UpT, U, nat[0], cc_ar_in, 0, 2, True)
            partial_stream(TarT, E, nat[1], cc_ar_in, U, 2, True)
            nc.gpsimd.collective_compute(
                "AllReduce",
                ADD,
                replica_groups=RG,
                ins=[cc_ar_in[:].opt()],
                outs=[cc_ar_out[:].opt()],
            )

            # ---- B3: geo partial + ReduceScatter ------------------------
            geo_part = constp.tile([D, P], FP8, name="geo_part")
            partial_stream(GeoT, P, nat[2], geo_part, 0, 4, False)
            nc.gpsimd.dma_start(
                cc_rs_in[:].rearrange("(r p) c -> p r c", p=128),
                geo_part[:].rearrange("p (r c) -> p r c", r=NCORES),
            )
            nc.gpsimd.collective_compute(
                "ReduceScatter",
                ADD,
                replica_groups=RG,
                ins=[cc_rs_in[:].opt()],
                outs=[cc_rs_out[:].opt()],
            )

            # ---- AllReduce readback: payload IS natural layout ----------
            yu_nat = constp.tile([128, U // 128, 128], FP8, name="yu_nat")
            yt_nat = constp.tile([128, E // 128, 128], FP8, name="yt_nat")
            nc.gpsimd.dma_start(
                yu_nat[:].rearrange("p a f -> p (a f)"), cc_ar_out[:, :U]
            )
            nc.gpsimd.dma_start(
                yt_nat[:].rearrange("p a f -> p (a f)"), cc_ar_out[:, U:]
            )

            # ---- row-sharded delta stream (C1 / C2) ---------------------
            def delta_stream(matT, n_k, lhs):
                kt_per = n_k // 2
                chunks = []
                for a in range(2):
                    ch = matp.tile([128, kt_per, PP], FP8, tag="mat")
                    eng = nc.sync if a % 2 == 0 else nc.scalar
                    eng.dma_start(
                        ch[:],
                        matT[a * kt_per * 128 : (a + 1) * kt_per * 128, :].rearrange(
                            "(a p) n -> p a n", p=128
                        ),
                    )
                    chunks.append(ch)
                ps = psg.tile([D, PP], F32, tag="psg")
                for j in range(n_k // 2):
                    a, jj = (2 * j) // kt_per, (2 * j) % kt_per
                    for h in range(2):
                        nc.tensor.matmul(
                            ps[:, h * 512 : h * 512 + 512],
                            lhs[:, 2 * j : 2 * j + 2, :],
                            chunks[a][:, jj : jj + 2, h * 512 : h * 512 + 512],
                            start=(j == 0),
                            stop=(j == n_k // 2 - 1),
                            perf_mode=DR,
                        )
                return ps

            def make_nat(srcT, dst):
                for j in range(2):
                    pst = pstr.tile([128, 512], F32, tag="pstr")
                    for m in range(4):
                        c = (4 * j + m) * 128
                        nc.tensor.transpose(
                            pst[:, m * 128 : (m + 1) * 128],
                            srcT[:, c : c + 128],
                            sb_idf[:],
                        )
                    if j == 0:
                        nc.vector.tensor_copy(dst[:, 4 * j : 4 * j + 4, :], pst[:])
                    else:
                        nc.scalar.activation(
                            dst[:, 4 * j : 4 * j + 4, :], pst[:], COPY
                        )

            # C1: hg_pois = x + Pu @ y_up
            ps_hg = delta_stream(PuT, U // 128, yu_nat)
            hg_poisT = big32.tile([D, PP], F32, tag="big32", name="hg_poisT")
            nc.vector.scalar_tensor_tensor(
                hg_poisT[:], ps_hg[:], SHG, ownT[0][:], MULT, ADD
            )
            nc.gpsimd.dma_start(poisT_o[0], hg_poisT[:])
            hg_nat = constp.tile([128, KL, 128], BF16, name="hg_nat")
            make_nat(hg_poisT, hg_nat)

            # C2: trans_pois = s + Src @ y_tar
            ps_tr = delta_stream(SrcT, E // 128, yt_nat)
            trans_poisT = big32.tile([D, PP], F32, tag="big32", name="trans_poisT")
            nc.vector.scalar_tensor_tensor(
                trans_poisT[:], ps_tr[:], SHG, ownT[1][:], MULT, ADD
            )
            nc.sync.dma_start(poisT_o[2], trans_poisT[:])

            # ---- geo finalize (ReduceScatter output) --------------------
            geo_sum = stagep.tile([D, PP], FP8, tag="sig", name="geo_sum")
            nc.gpsimd.dma_start(geo_sum[:], cc_rs_out[:])
            geo_poisT = big32.tile([D, PP], F32, tag="big32", name="geo_poisT")
            nc.vector.scalar_tensor_tensor(
                geo_poisT[:], geo_sum[:], GEO_SCALE, ownT[2][:], MULT, ADD
            )
            nc.scalar.dma_start(poisT_o[1], geo_poisT[:])
            geo_nat = constp.tile([128, KL, 128], BF16, name="geo_nat")
            make_nat(geo_poisT, geo_nat)

            # ---- D: user partials (bf16, host reduces) ------------------
            ps_hu = psg.tile([D, B], F32, tag="psg")
            ps_gu = psg.tile([D, B], F32, tag="psg")
            for k in range(KL):
                for h in range(2):
                    cols = slice(h * 512, h * 512 + 512)
                    nc.tensor.matmul(
                        ps_hu[:, cols], hg_nat[:, k, :], sel[:, k, cols],
                        start=(k == 0), stop=(k == KL - 1),
                    )
                    nc.tensor.matmul(
                        ps_gu[:, cols], geo_nat[:, k, :], sel[:, k, cols],
                        start=(k == 0), stop=(k == KL - 1),
                    )
            users_sb = outp.tile([D, 2 * B], F32, name="users_sb")
            nc.vector.tensor_copy(users_sb[:, :B], ps_hu[:])
            nc.scalar.activation(users_sb[:, B:], ps_gu[:], COPY)
            nc.sync.dma_start(usersT_o[:, :B], users_sb[:, :B])
            nc.scalar.dma_start(usersT_o[:, B:], users_sb[:, B:])

    nc.compile()
    return nc


def _get_nc():
    if "nc" not in _CACHE:
        _CACHE["nc"] = _build_nc()
    return _CACHE["nc"]


def _shard_inputs(inputs):
    f32 = np.float32
    bf16 = ml_dtypes.bfloat16
    fp8 = ml_dtypes.float8_e4m3
    pe = np.asarray(inputs["poi_emb_weight"], f32)[:P]
    peT = np.ascontiguousarray(pe.T)                     # [D, P]
    w3 = np.stack(
        [
            np.asarray(inputs["w_gate_col"], f32),
            np.asarray(inputs["w_gate_seq"], f32),
            np.asarray(inputs["w_gate_geo"], f32),
        ],
        axis=1,
    ).astype(bf16)                                        # [D, 3, D]
    bT3 = np.stack(
        [
            np.asarray(inputs["b_gate_col"], f32)[0],
            np.asarray(inputs["b_gate_seq"], f32)[0],
            np.asarray(inputs["b_gate_geo"], f32)[0],
        ],
        axis=1,
    )
    eye = np.eye(D, dtype=f32)
    idx = np.asarray(inputs["user_idx"]).astype(np.int64)
    Up = np.asarray(inputs["HG_up"], f32)                 # [U, P]
    Pu = np.asarray(inputs["HG_pu"], f32)                 # [P, U]
    Tar = np.asarray(inputs["HG_poi_tar"], f32)           # [E, P]
    Src = np.asarray(inputs["HG_poi_src"], f32)           # [P, E]
    Geo = np.asarray(inputs["poi_geo_graph"], f32)        # [P, P]
    Sel = Up[idx]                                         # [B, P]
    S18, S17 = 2.0 ** 18, 2.0 ** 17

    in_maps = []
    for i in range(NCORES):
        rp = slice(PP * i, PP * (i + 1))
        in_maps.append(
            {
                "peT_bf": peT[:, rp].astype(bf16),
                "peT_f": np.ascontiguousarray(peT[:, rp]),
                "w3": w3,
                "bT3": bT3,
                "ident_f": eye,
                "ident_b": eye.astype(bf16),
                "UpT": (Up.T[rp] * S18).astype(fp8),
                "TarT": (Tar.T[rp] * S18).astype(fp8),
                "GeoT": (Geo.T[rp] * S18).astype(fp8),
                "PuT": (Pu[rp].T * S17).astype(fp8),
                "SrcT": (Src[rp].T * S17).astype(fp8),
                "SelT": np.ascontiguousarray(Sel[:, rp].T).astype(bf16),
            }
        )
    return in_maps


def _assemble(results):
    f32 = np.float32
    hg = np.empty((P, D), f32)
    geo = np.empty((P, D), f32)
    tr = np.empty((P, D), f32)
    users = np.zeros((D, 2 * B), f32)
    for i in range(NCORES):
        rp = slice(PP * i, PP * (i + 1))
        pois = results[i]["poisT_o"]
        hg[rp] = pois[0].T
        geo[rp] = pois[1].T
        tr[rp] = pois[2].T
        users += results[i]["usersT_o"]
    return np.concatenate([hg, geo, tr, users[:, :B].T, users[:, B:].T], axis=0)


def _run(inputs, trace=False, **spmd_kwargs):
    nc = _get_nc()
    in_maps = _shard_inputs(inputs)
    res = run_bass_kernel_spmd(
        nc, in_maps, list(range(NCORES)), trace=trace, **spmd_kwargs
    )
    return _assemble(res.results), res


def kernel(**inputs):
    return _run(inputs)[0]


if __name__ == "__main__":
    import pickle

    with open("/tmp/inputs.pkl", "rb") as f:
        inputs = pickle.load(f)
    out = kernel(**inputs)
    exp = np.load("/tmp/expected.npy")
    rel = np.linalg.norm(out - exp) / np.linalg.norm(exp)
    print("Relative error:", rel)
